# revision 1
# baseline (speedup 1.0000x reference)
"""Trainium2 kernel for nn_DEC_2619930050954 (sparse_attention).

Strategy (per sharding hint): pure data parallel — shard the batch dim of
`received` (512) across the 8 NeuronCores (64 per core), replicate all
RNN/attention/fc weights on every core. The sequential GRU scan stays local
per device; the sliding-window attention, fc, delay-align and decode head are
all computed on-device per shard; outputs are gathered to the full [512,100,1].
"""

import jax
import jax.numpy as jnp
import numpy as np

B, L, NIN, H, D = 512, 100, 3, 256, 10
WIN = 5
NCORES = 8
BS = B // NCORES  # 64 per core


def _gru_layer(x, Wih, Whh, bih, bhh):
    # x: [b, L, in]; PyTorch GRU gate math, gates ordered (r, z, n)
    xw = jnp.einsum('bli,gi->blg', x, Wih) + bih  # [b, L, 3H]
    h0 = jnp.zeros((x.shape[0], Whh.shape[1]), x.dtype)

    def step(h, xt):
        gh = h @ Whh.T + bhh
        xr, xz, xn = jnp.split(xt, 3, axis=-1)
        hr, hz, hn = jnp.split(gh, 3, axis=-1)
        r = jax.nn.sigmoid(xr + hr)
        z = jax.nn.sigmoid(xz + hz)
        n = jnp.tanh(xn + r * hn)
        h_new = (1.0 - z) * n + z * h
        return h_new, h_new

    _, ys = jax.lax.scan(step, h0, jnp.swapaxes(xw, 0, 1))
    return jnp.swapaxes(ys, 0, 1)  # [b, L, H]


def _gru2(x, Wih0, Whh0, bih0, bhh0, Wih1, Whh1, bih1, bhh1):
    h1 = _gru_layer(x, Wih0, Whh0, bih0, bhh0)
    return _gru_layer(h1, Wih1, Whh1, bih1, bhh1)


def _branch(out, attn_w, fc_W, fc_b):
    # Sliding-window attention over all timesteps i >= WIN.
    q = out @ attn_w[:H]   # [b, L]
    e = out @ attn_w[H:]   # [b, L]
    M = L - WIN
    win = jnp.stack([out[:, k:k + M] for k in range(WIN)], axis=2)  # [b, M, 5, H]
    ewin = jnp.stack([e[:, k:k + M] for k in range(WIN)], axis=2)   # [b, M, 5]
    scores = q[:, WIN:, None] + ewin
    a = jax.nn.softmax(scores, axis=-1)
    c = jnp.einsum('blw,blwh->blh', a, win)                          # [b, M, H]
    fused = jnp.concatenate([c, out[:, WIN:]], axis=-1) @ fc_W.T + fc_b
    return jnp.concatenate([out[:, :WIN], fused], axis=1)            # [b, L, H]


def _model(received,
           r1_Wih0, r1_Whh0, r1_bih0, r1_bhh0, r1_Wih1, r1_Whh1, r1_bih1, r1_bhh1,
           r2_Wih0, r2_Whh0, r2_bih0, r2_bhh0, r2_Wih1, r2_Whh1, r2_bih1, r2_bhh1,
           attn_w, fc_W, fc_b, out_W, out_b):
    out1 = _gru2(received, r1_Wih0, r1_Whh0, r1_bih0, r1_bhh0,
                 r1_Wih1, r1_Whh1, r1_bih1, r1_bhh1)
    out2 = _gru2(received, r2_Wih0, r2_Whh0, r2_bih0, r2_bhh0,
                 r2_Wih1, r2_Whh1, r2_bih1, r2_bhh1)
    rnn_out1 = _branch(out1, attn_w, fc_W, fc_b)
    rnn_out2 = _branch(out2, attn_w, fc_W, fc_b)
    idx = jnp.minimum(jnp.arange(L) + D, L - 1)
    rt_d = rnn_out2[:, idx]
    rnn_out = jnp.concatenate([rnn_out1, rt_d], axis=-1)  # [b, L, 2H]
    dec = rnn_out @ out_W.T + out_b
    return jax.nn.sigmoid(dec)                            # [b, L, 1]


_ORDER = ['received',
          'r1_Wih0', 'r1_Whh0', 'r1_bih0', 'r1_bhh0',
          'r1_Wih1', 'r1_Whh1', 'r1_bih1', 'r1_bhh1',
          'r2_Wih0', 'r2_Whh0', 'r2_bih0', 'r2_bhh0',
          'r2_Wih1', 'r2_Whh1', 'r2_bih1', 'r2_bhh1',
          'attn_w', 'fc_W', 'fc_b', 'out_W', 'out_b']

_pmapped = jax.pmap(_model, in_axes=(0,) + (None,) * 21)


def kernel(**inputs: np.ndarray) -> np.ndarray:
    args = [np.asarray(inputs[k]) for k in _ORDER]
    received = args[0].astype(np.float32)
    # shard batch across the 8 cores
    recv_sh = received.reshape(NCORES, BS, L, NIN)
    out_sh = _pmapped(recv_sh, *args[1:])
    out = np.asarray(out_sh).reshape(B, L, 1).astype(np.float32)
    return out
